# revision 7
# baseline (speedup 1.0000x reference)
"""Trainium2 Bass kernel for nn_Compensation_20220706029991 (2-layer GCN + input MLP).

Math (with A_hat = D^-1/2 (A+I) D^-1/2, all feature dims 256):
  h0  = drop1(relu(omega @ W_lin + b_lin))
  h1  = drop2(relu(A_hat (h0 @ W_g1) + b_g1))
  h2  = A_hat (h1 @ W_g2) + b_g2
  out = z * h2

Since (A+I) X W == ((A+I) X) W, we push the D^-1/2 scalings into the node
features:  hs = dinv * h;  agg = (A+I) hs;  g = dinv * agg;  next = f(g @ W).

Distribution: nodes are sharded across 8 NeuronCores (12500 rows each,
padded to 12544 = 98 buckets of 128).  Weights are replicated.  The
scatter-add aggregation runs as: per-core dma_gather of source rows (bf16)
from a full replicated table in local HBM, then a one-hot matmul scatter
into PSUM per 128-row destination bucket.  The full table is produced by an
on-device AllGather between layers (the cross-partition "halo exchange").

Everything data-dependent (degrees, edge sort, padding layout) is host-side
numpy preprocessing; the NEFF structure depends only on per-(bucket,chunk)
block counts shared by all cores.
"""
import numpy as np
import ml_dtypes

N = 100000
E_DECL = 1600000
D = 256
NCORES = 8
P = 128
NRP = 12500                # owned rows per core
NBUK = 98                  # buckets of 128 dst rows (12544 padded)
NRPAD = NBUK * P           # 12544
TROWS = NCORES * NRPAD     # 100352 rows in the gathered table
NCHUNK = 4
CH = TROWS // NCHUNK       # 25088 (< 32767, int16-addressable)
TGB = 7                    # max blocks (of 128 idxs) per dma_gather call
DMA_SCRATCH = 32768        # SWDGE ring = 2048 descriptors
DROP_SCALE = 2.0           # 1/(1-0.5)

bf16 = ml_dtypes.bfloat16

_CACHE = {}


def _preprocess(edge_index):
    """Build per-core padded edge streams. Returns host arrays + structure."""
    src = np.asarray(edge_index[0], dtype=np.int64)
    dst = np.asarray(edge_index[1], dtype=np.int64)

    deg = np.bincount(dst, minlength=N).astype(np.float64) + 1.0
    dinv = (1.0 / np.sqrt(deg)).astype(np.float32)

    # append self loops
    loop = np.arange(N, dtype=np.int64)
    src = np.concatenate([src, loop])
    dst = np.concatenate([dst, loop])

    core = dst // NRP
    rloc = dst - core * NRP
    bucket = rloc >> 7
    dpos = rloc & 127                       # dst position within bucket
    trow = (src // NRP) * NRPAD + (src % NRP)
    chunk = trow // CH
    cidx = (trow - chunk * CH).astype(np.int64)

    # lexsort per (core, chunk, bucket) , then by cidx for HBM locality
    order = np.lexsort((cidx, bucket, chunk, core))
    core_s = core[order]
    chunk_s = chunk[order]
    bucket_s = bucket[order]
    dpos_s = dpos[order]
    cidx_s = cidx[order]

    # segment id = ((core*NCHUNK + chunk)*NBUK + bucket)
    seg = (core_s * NCHUNK + chunk_s) * NBUK + bucket_s
    nseg = NCORES * NCHUNK * NBUK
    cnt = np.bincount(seg, minlength=nseg).reshape(NCORES, NCHUNK, NBUK)

    # shared block counts: NB[k][b] = max over cores ceil(cnt/128)
    NB = np.ceil(cnt.max(axis=0) / P).astype(np.int64)          # [NCHUNK, NBUK]
    nbcum = np.zeros((NCHUNK, NBUK), np.int64)                  # block offset of (k,b) within chunk stream
    for k in range(NCHUNK):
        nbcum[k] = np.cumsum(NB[k]) - NB[k]
    nbk = NB.sum(axis=1)                                        # blocks per chunk
    colbase = np.concatenate([[0], np.cumsum(nbk)])             # global block col base per chunk
    nbtot = int(colbase[-1])

    # per-edge position within its padded stream
    # rank within segment:
    seg_sorted = seg  # already grouped
    starts = np.concatenate([[0], np.cumsum(np.bincount(seg, minlength=nseg))])
    rank = np.arange(seg.shape[0], dtype=np.int64) - starts[seg_sorted]
    # position within chunk stream (per core): block col within chunk = nbcum[k,b], so
    pos_in_chunk = (nbcum[chunk_s, bucket_s] * P + rank)
    gcol = colbase[chunk_s] + pos_in_chunk // P                 # global block col
    gpart = pos_in_chunk & 127

    # fill idx + dstloc arrays per core
    idx_arr = np.zeros((NCORES, nbtot * P), np.int16)           # flat by (gcol, part)
    dst_arr = np.full((NCORES, nbtot * P), -1.0, np.float32)
    flat = gcol * P + gpart
    idx_arr[core_s, flat] = cidx_s.astype(np.int16)
    dst_arr[core_s, flat] = dpos_s.astype(np.float32)

    # wrapped int16 layout [128, nbtot*8]: within chunk k the cols are
    # contiguous: col c of the wrapped array = position run s*16 + (p%16).
    # global wrapped col for chunk-stream position i: colbase[k]*8 + i//16.
    idx_wrap = np.zeros((NCORES, P, nbtot * 8), np.int16)
    s_idx = np.arange(nbtot * 8)
    for p_mod in range(16):
        cols = idx_arr[:, s_idx * 16 + p_mod]                   # [NCORES, nbtot*8]
        idx_wrap[:, p_mod::16, :] = cols[:, None, :]
    # dstloc [128, nbtot] (bf16): column = global block col
    dstloc = dst_arr.reshape(NCORES, nbtot, P).transpose(0, 2, 1).astype(bf16)

    # gather call list per chunk: runs of <= TGB blocks
    calls = []                                                   # (k, i0, nb)
    for k in range(NCHUNK):
        i = 0
        while i < nbk[k]:
            nb = min(TGB, nbk[k] - i)
            calls.append((k, int(i), int(nb)))
            i += nb
    # map chunk block index -> (call#, block-within-call)
    blockmap = {}
    for ci, (k, i0, nb) in enumerate(calls):
        for j in range(nb):
            blockmap[(k, i0 + j)] = (ci, j)

    struct = dict(NB=NB, nbcum=nbcum, nbk=nbk, colbase=colbase, nbtot=nbtot,
                  calls=calls, blockmap=blockmap)
    return dinv, idx_wrap, dstloc, struct


def _build_program(struct):
    from concourse import bacc, mybir, tile
    from concourse import library_config

    NB = struct["NB"]; nbcum = struct["nbcum"]; colbase = struct["colbase"]
    nbtot = struct["nbtot"]; calls = struct["calls"]; blockmap = struct["blockmap"]

    f32 = mybir.dt.float32
    b16 = mybir.dt.bfloat16

    nc = bacc.Bacc(None, target_bir_lowering=False, debug=False,
                   dynamic_dma_scratch_size=DMA_SCRATCH)

    p_omT = nc.declare_dram_parameter("omT", [NBUK, 2, P, P], b16, isOutput=False)
    p_M1 = nc.declare_dram_parameter("M1", [NRPAD, D], f32, isOutput=False)
    p_M2 = nc.declare_dram_parameter("M2", [NRPAD, D], f32, isOutput=False)
    p_z = nc.declare_dram_parameter("z", [NRPAD, D], f32, isOutput=False)
    p_W = nc.declare_dram_parameter("W", [3, 2, P, D], b16, isOutput=False)
    p_b = nc.declare_dram_parameter("b", [3, 1, D], b16, isOutput=False)
    p_dinv = nc.declare_dram_parameter("dinvc", [P, NBUK], f32, isOutput=False)
    p_iota = nc.declare_dram_parameter("iota", [P, P], b16, isOutput=False)
    p_ident = nc.declare_dram_parameter("ident", [P, P], b16, isOutput=False)
    p_ones = nc.declare_dram_parameter("ones", [1, P], b16, isOutput=False)
    p_idx = nc.declare_dram_parameter("idxw", [P, nbtot * 8], mybir.dt.int16, isOutput=False)
    p_dstloc = nc.declare_dram_parameter("dstloc", [P, nbtot], b16, isOutput=False)
    p_out = nc.declare_dram_parameter("out", [NRPAD, D], f32, isOutput=True)

    with tile.TileContext(nc) as tc:
        with tc.tile_pool(name="const", bufs=1) as constp, \
             tc.tile_pool(name="dram", bufs=1, space="DRAM") as dram, \
             tc.tile_pool(name="omt", bufs=3) as omtp, \
             tc.tile_pool(name="scale", bufs=3) as scalep, \
             tc.tile_pool(name="oh", bufs=3) as ohp, \
             tc.tile_pool(name="g", bufs=3) as gp, \
             tc.tile_pool(name="gT", bufs=3) as gTp, \
             tc.tile_pool(name="ot", bufs=3) as otp, \
             tc.tile_pool(name="m0", bufs=3) as m0p, \
             tc.tile_pool(name="m1", bufs=3) as m1p, \
             tc.tile_pool(name="m2", bufs=3) as m2p, \
             tc.tile_pool(name="m3", bufs=3) as m3p, \
             tc.tile_pool(name="psA", bufs=2, space="PSUM") as psA, \
             tc.tile_pool(name="psT", bufs=2, space="PSUM") as psT, \
             tc.tile_pool(name="psD", bufs=2, space="PSUM") as psD:
            msgp = [m0p, m1p, m2p, m3p]
            nc.gpsimd.load_library(library_config.mlp)

            # ---------- resident constants ----------
            iota_t = constp.tile([P, P], b16)
            nc.sync.dma_start(out=iota_t[:], in_=p_iota[:])
            ident_t = constp.tile([P, P], b16)
            nc.sync.dma_start(out=ident_t[:], in_=p_ident[:])
            ones_t = constp.tile([1, P], b16)
            nc.sync.dma_start(out=ones_t[:], in_=p_ones[:])
            dinv_t = constp.tile([P, NBUK], f32)
            nc.sync.dma_start(out=dinv_t[:], in_=p_dinv[:])
            W_t = constp.tile([P, 3 * 2 * D], b16)
            for li in range(3):
                for h in range(2):
                    nc.sync.dma_start(out=W_t[:, (li * 2 + h) * D:(li * 2 + h + 1) * D],
                                      in_=p_W[li, h])
            b_t = constp.tile([1, 3 * D], b16)
            for li in range(3):
                nc.sync.dma_start(out=b_t[:, li * D:(li + 1) * D], in_=p_b[li])
            dstloc_t = constp.tile([P, nbtot], b16)
            nc.sync.dma_start(out=dstloc_t[:], in_=p_dstloc[:])
            idx_t = constp.tile([P, nbtot * 8], mybir.dt.int16)
            nc.sync.dma_start(out=idx_t[:], in_=p_idx[:])

            # DRAM internals
            hs_part = dram.tile([NRPAD, D], b16, tag="hs_part")       # this core's slice
            hs_full = dram.tile([TROWS, D], b16, tag="hs_full")       # gathered table
            hs_part2 = dram.tile([NRPAD, D], b16, tag="hs_part2")
            hs_full2 = dram.tile([TROWS, D], b16, tag="hs_full2")

            def Wsl(li, h):
                return W_t[:, (li * 2 + h) * D:(li * 2 + h + 1) * D]

            # ---------- stage A: layer 0 on owned rows ----------
            for b in range(NBUK):
                omt = omtp.tile([P, 2, P], b16, tag="omt")
                for h in range(2):
                    nc.scalar.dma_start(out=omt[:, h, :], in_=p_omT[b, h])
                m1t = scalep.tile([P, D], f32, tag="scl")
                nc.scalar.dma_start(out=m1t[:], in_=p_M1[b * P:(b + 1) * P, :])
                ps = psA.tile([P, D], f32, space="PSUM", tag="psa")
                for h in range(2):
                    nc.tensor.matmul(out=ps[:], lhsT=omt[:, h, :], rhs=Wsl(0, h),
                                     start=(h == 0), stop=False)
                nc.tensor.matmul(out=ps[:], lhsT=ones_t[:], rhs=b_t[:, 0:D],
                                 start=False, stop=True)
                hs0 = gp.tile([P, D], b16, tag="g")
                nc.vector.scalar_tensor_tensor(
                    out=hs0[:], in0=ps[:], scalar=0.0, in1=m1t[:],
                    op0=mybir.AluOpType.max, op1=mybir.AluOpType.mult)
                nc.sync.dma_start(out=hs_part[b * P:(b + 1) * P, :], in_=hs0[:])

            # ---------- AllGather hs0 ----------
            nc.gpsimd.collective_compute(
                "AllGather", mybir.AluOpType.bypass,
                replica_groups=[list(range(NCORES))],
                ins=[hs_part[:].opt()], outs=[hs_full[:].opt()])

            # ---------- SpMM + dense layer (shared for layers 1, 2) ----------
            def spmm_layer(table, li, out_cb):
                """agg = (A+I) table ; g = dinv*agg ; y = g @ W[li] + b[li] ; out_cb."""
                call_tiles = [None] * len(calls)

                def ensure_call(ci):
                    if call_tiles[ci] is not None:
                        return call_tiles[ci]
                    k, i0, nb = calls[ci]
                    mt = msgp[k].tile([P, TGB, D], b16, tag=f"msg{k}")
                    wc0 = (colbase[k] + i0) * 8
                    nc.gpsimd.dma_gather(
                        out_ap=mt[:, 0:nb, :],
                        in_ap=table[k * CH:(k + 1) * CH, :],
                        idxs_ap=idx_t[:, wc0:wc0 + nb * 8],
                        num_idxs=nb * P, num_idxs_reg=nb * P, elem_size=D)
                    call_tiles[ci] = mt
                    return mt

                for b in range(NBUK):
                    ps = psA.tile([P, D], f32, space="PSUM", tag="psa")
                    blocks = []
                    for k in range(NCHUNK):
                        for j in range(int(NB[k][b])):
                            blocks.append((k, int(nbcum[k][b]) + j))
                    for bi, (k, cblk) in enumerate(blocks):
                        ci, jin = blockmap[(k, cblk)]
                        mt = ensure_call(ci)
                        gc = colbase[k] + cblk
                        oh = ohp.tile([P, P], b16, tag="oh")
                        nc.vector.tensor_tensor(
                            out=oh[:], in0=dstloc_t[:, gc:gc + 1].to_broadcast([P, P]),
                            in1=iota_t[:], op=mybir.AluOpType.is_equal)
                        nc.tensor.matmul(out=ps[:], lhsT=oh[:], rhs=mt[:, jin, :],
                                         start=(bi == 0), stop=(bi == len(blocks) - 1))
                    # g = dinv * agg  (bf16)
                    g_t = gp.tile([P, D], b16, tag="g")
                    nc.vector.tensor_scalar(out=g_t[:], in0=ps[:],
                                            scalar1=dinv_t[:, b:b + 1], scalar2=None,
                                            op0=mybir.AluOpType.mult)
                    # transpose
                    tp = psT.tile([P, D], b16, space="PSUM", tag="pst")
                    for h in range(2):
                        nc.tensor.transpose(out=tp[:, h * P:(h + 1) * P],
                                            in_=g_t[:, h * P:(h + 1) * P],
                                            identity=ident_t[:])
                    gT_t = gTp.tile([P, D], b16, tag="gT")
                    nc.vector.tensor_copy(out=gT_t[:], in_=tp[:])
                    # dense + bias
                    pd = psD.tile([P, D], f32, space="PSUM", tag="psd")
                    for h in range(2):
                        nc.tensor.matmul(out=pd[:], lhsT=gT_t[:, h * P:(h + 1) * P],
                                         rhs=Wsl(li, h), start=(h == 0), stop=False)
                    nc.tensor.matmul(out=pd[:], lhsT=ones_t[:], rhs=b_t[:, li * D:(li + 1) * D],
                                     start=False, stop=True)
                    out_cb(b, pd)

            # layer 1: epilogue relu * M2 -> hs_part2
            def l1_out(b, pd):
                m2t = scalep.tile([P, D], f32, tag="scl")
                nc.scalar.dma_start(out=m2t[:], in_=p_M2[b * P:(b + 1) * P, :])
                hs1 = otp.tile([P, D], b16, tag="hs1")
                nc.vector.scalar_tensor_tensor(
                    out=hs1[:], in0=pd[:], scalar=0.0, in1=m2t[:],
                    op0=mybir.AluOpType.max, op1=mybir.AluOpType.mult)
                nc.sync.dma_start(out=hs_part2[b * P:(b + 1) * P, :], in_=hs1[:])

            spmm_layer(hs_full, 1, l1_out)

            # ---------- AllGather hs1 ----------
            nc.gpsimd.collective_compute(
                "AllGather", mybir.AluOpType.bypass,
                replica_groups=[list(range(NCORES))],
                ins=[hs_part2[:].opt()], outs=[hs_full2[:].opt()])

            # layer 2: epilogue * z -> out
            def l2_out(b, pd):
                zt = scalep.tile([P, D], f32, tag="scl")
                nc.scalar.dma_start(out=zt[:], in_=p_z[b * P:(b + 1) * P, :])
                o_t = otp.tile([P, D], f32, tag="ofin")
                nc.vector.tensor_tensor(out=o_t[:], in0=pd[:], in1=zt[:],
                                        op=mybir.AluOpType.mult)
                nc.sync.dma_start(out=p_out[b * P:(b + 1) * P, :], in_=o_t[:])

            spmm_layer(hs_full2, 2, l2_out)

    nc.compile()
    return nc


def _prep_inputs(z, omega, edge_index, W_lin, b_lin, W_g1, b_g1, W_g2, b_g2,
                 mask1, mask2):
    dinv, idx_wrap, dstloc, struct = _preprocess(np.asarray(edge_index))

    z = np.asarray(z, np.float32)
    omega = np.asarray(omega, np.float32)
    mask1 = np.asarray(mask1)
    mask2 = np.asarray(mask2)

    Wst = np.stack([np.asarray(W_lin, np.float32),
                    np.asarray(W_g1, np.float32),
                    np.asarray(W_g2, np.float32)]).astype(bf16)  # [3, 256, 256]
    Wst = Wst.reshape(3, 2, P, D)
    bst = np.stack([np.asarray(b_lin, np.float32),
                    np.asarray(b_g1, np.float32),
                    np.asarray(b_g2, np.float32)]).astype(bf16).reshape(3, 1, D)
    iota = np.broadcast_to(np.arange(P, dtype=np.float32), (P, P)).astype(bf16)
    ident = np.eye(P, dtype=np.float32).astype(bf16)
    ones = np.ones((1, P), np.float32).astype(bf16)

    in_maps = []
    for c in range(NCORES):
        sl = slice(c * NRP, (c + 1) * NRP)
        dinv_c = dinv[sl]
        omT_pad = np.zeros((D, NRPAD), np.float32)
        omT_pad[:, :NRP] = omega[sl].T
        # tiled [NBUK, 2, P, P]: omT[b, h] = omega.T[h*128:(h+1)*128, b*128:(b+1)*128]
        omT = np.ascontiguousarray(
            omT_pad.reshape(2, P, NBUK, P).transpose(2, 0, 1, 3)).astype(bf16)
        M1 = np.zeros((NRPAD, D), np.float32)
        M1[:NRP] = np.where(mask1[sl], DROP_SCALE * dinv_c[:, None], 0.0)
        M2 = np.zeros((NRPAD, D), np.float32)
        M2[:NRP] = np.where(mask2[sl], DROP_SCALE * dinv_c[:, None], 0.0)
        zc = np.zeros((NRPAD, D), np.float32)
        zc[:NRP] = z[sl]
        dinv_pad = np.ones(NRPAD, np.float32)
        dinv_pad[:NRP] = dinv_c
        dinvc = np.ascontiguousarray(dinv_pad.reshape(NBUK, P).T)  # [P, NBUK]
        in_maps.append({
            "omT": omT, "M1": M1, "M2": M2, "z": zc, "W": Wst, "b": bst,
            "dinvc": dinvc, "iota": iota, "ident": ident, "ones": ones,
            "idxw": idx_wrap[c], "dstloc": dstloc[c],
        })
    return in_maps, struct


def _get_program_and_inputs(**inputs):
    in_maps, struct = _prep_inputs(**inputs)
    key = (struct["nbtot"], tuple(struct["nbk"]))
    if key not in _CACHE:
        _CACHE[key] = _build_program(struct)
    return _CACHE[key], in_maps


def kernel(**inputs):
    from concourse.bass_utils import run_bass_kernel_spmd
    nc, in_maps = _get_program_and_inputs(**inputs)
    res = run_bass_kernel_spmd(nc, in_maps, core_ids=list(range(NCORES)))
    out = np.empty((N, D), np.float32)
    for c in range(NCORES):
        out[c * NRP:(c + 1) * NRP] = res.results[c]["out"][:NRP]
    return out
